# revision 24
# baseline (speedup 1.0000x reference)
"""Trainium2 Bass kernel for BatchEmbeddingUpdater (GNN message passing).

Contract: kernel(**inputs) takes the FULL inputs (as produced by the
reference setup_inputs()) and returns the FULL outputs
(updated_src_table, updated_dst_table), each [200000, 128] f32.

Sharding strategy (8 cores):
  - Both node-embedding tables are sharded row-block-wise over the
    non-updated region [BATCH, N_NODES); each core copies its shard
    input->output on device (HBM->HBM DMA) - the memory-bound bulk.
    The batch rows' old values reach the device as gather inputs and
    their new values come back as compute outputs, so copying them too
    would be redundant traffic.
  - The 8192-row batch is sharded by batch position: core i computes batch
    rows [1024*i, 1024*(i+1)) for BOTH sides. The host routes the gathered
    previous-embedding rows for those batch positions to core i (pre
    transposed to [128, 1024] so the device needs no transposes), the core
    runs the two-layer MLP, and returns the updated rows transposed
    [128, 1024]. The host scatters them into the assembled output.
  - The small linear weights are replicated to every core (packed into a
    single [128, 1029] tensor per side so one DMA loads them).

All DMA rides the sync (SP) HWDGE ring in FIFO order: the 4 input loads
first, then the shard-copy chunks with the updT stores interleaved
between them so the stores drain mid-stream instead of behind 24.5MB of
copy descriptors.
"""

import numpy as np

import concourse.bass as bass
import concourse.tile as tile
from concourse import mybir
from concourse.bass_utils import run_bass_kernel_spmd


def _split_multi_waits(nc, max_waits=1):
    """The walrus build in this image rejects multiple sem waits on one
    instruction ("Too many sync wait commands"). Move excess waits onto
    single-wait NOPs inserted just before the instruction on the same
    engine (per-engine program order makes this equivalent)."""
    ctr = 0
    for fn in nc.m.functions:
        for blk in fn.blocks:
            new_insts = []
            changed = False
            for ins in blk.instructions:
                si = ins.sync_info
                waits = list(si.on_wait) if si is not None else []
                if len(waits) > max_waits:
                    changed = True
                    for i in range(max_waits, len(waits), max_waits):
                        nop = mybir.InstNoOp(
                            name=f"I-waitsplit-{ctr}",
                            engine=ins.engine,
                            sync_info=mybir.SyncInfo(
                                on_wait=waits[i:i + max_waits], on_update=[]),
                        )
                        ctr += 1
                        new_insts.append(nop)
                    ins.sync_info = mybir.SyncInfo(
                        on_wait=waits[:max_waits],
                        on_update=list(si.on_update))
                new_insts.append(ins)
            if changed:
                blk.instructions = new_insts


N_CORES = 8
N_NODES = 200000
BATCH = 8192
ROWS = (N_NODES - BATCH) // N_CORES  # 23976 copied rows per core
DIM = 128                  # node/nig embedding dim
HID = 256                  # hidden dim
BSL = BATCH // N_CORES     # 1024 batch rows per core
BCHUNK = 512               # batch columns per matmul (one PSUM bank)
WCOLS = 2 * HID + 4 * DIM + 4 + 1  # packed weights: 1029 cols

# Shard-copy descriptor scheme. SDMA engine 15 of the HWDGE ring runs
# ~18% slower than the other 15 (queue bookkeeping rides its AXI path),
# and descriptors are dealt round-robin over the 16 engine slots, so a
# uniform split leaves engine 15 straggling ~10us after everyone else.
# Every copy "pair" is a 15-descriptor DMA (prime descriptor size X
# forces exactly 15) followed by a 1-descriptor DMA (prime Y), so one
# fixed slot always receives the smaller descriptor. With Y/X ~ 0.82 the
# slow engine finishes with the rest. All other DMAs have descriptor
# counts divisible by 16, keeping the deal phase constant.
CP_X = 16001               # elems per big descriptor (prime, 64004B)
CP_Y = 13183               # elems per small descriptor (prime, 52732B)
CP_PAIR = 15 * CP_X + CP_Y  # 253198 elems per pair
CP_NPAIRS = 12             # pairs per table side
CP_REM = ROWS * DIM - CP_NPAIRS * CP_PAIR  # 30552 elems, copied at the end

F32 = mybir.dt.float32
SIDES = ("src", "dst")

_CACHE: dict = {}


def _build_nc():
    nc = bass.Bass("TRN2", target_bir_lowering=False, debug=False,
                   num_devices=N_CORES)

    io = {}
    for s in SIDES:
        io[f"{s}_shard"] = nc.dram_tensor(
            f"{s}_shard", [ROWS * DIM], F32, kind="ExternalInput").ap()
        io[f"{s}_ins"] = nc.dram_tensor(
            f"{s}_ins", [DIM, WCOLS + 2 * BSL], F32,
            kind="ExternalInput").ap()
        io[f"{s}_out_shard"] = nc.dram_tensor(
            f"{s}_out_shard", [ROWS * DIM], F32, kind="ExternalOutput").ap()
        io[f"{s}_updT"] = nc.dram_tensor(
            f"{s}_updT", [DIM, BSL], F32, kind="ExternalOutput").ap()

    def copy_pair(s, p):
        o = p * CP_PAIR
        nc.sync.dma_start(out=io[f"{s}_out_shard"][o:o + 15 * CP_X],
                          in_=io[f"{s}_shard"][o:o + 15 * CP_X])
        nc.sync.dma_start(
            out=io[f"{s}_out_shard"][o + 15 * CP_X:o + CP_PAIR],
            in_=io[f"{s}_shard"][o + 15 * CP_X:o + CP_PAIR])

    def copy_rem(s):
        o = CP_NPAIRS * CP_PAIR
        nc.sync.dma_start(out=io[f"{s}_out_shard"][o:o + CP_REM],
                          in_=io[f"{s}_shard"][o:o + CP_REM])

    with tile.TileContext(nc) as tc:
        with (
            tc.tile_pool(name="const", bufs=1) as cpool,
            tc.tile_pool(name="acts", bufs=2) as apool,
            tc.tile_pool(name="outs", bufs=4) as opool,
            tc.tile_pool(name="psum_cat", bufs=1, space="PSUM") as pcat,
            tc.tile_pool(name="psum_out", bufs=2, space="PSUM") as pout,
        ):
            cons = {}
            for s in SIDES:
                t = cpool.tile([DIM, WCOLS + 2 * BSL], F32, tag=f"{s}_ins")
                nc.sync.dma_start(out=t[:], in_=io[f"{s}_ins"][:])
                cons[f"{s}_ins"] = t

            # feed the ring: 9 pairs per side up front, the rest after the
            # store stalls so the stream never runs dry
            for p in range(9):
                copy_pair("src", p)
                copy_pair("dst", p)

            def compute_side(s):
                w = cons[f"{s}_ins"][:, :WCOLS]
                x = cons[f"{s}_ins"][:, WCOLS:]
                out_sb = opool.tile([DIM, BSL], F32, tag="out_sb")
                for c in range(BSL // BCHUNK):
                    bs = bass.ts(c, BCHUNK)
                    # catT chunks: [sel0, sel1, shift0, shift1];
                    # chunk j covers hidden units [128j, 128(j+1))
                    cat_ps = pcat.tile([DIM, 4, BCHUNK], F32, tag="cat")
                    for j in range(4):
                        lhsT = w[:, j * DIM:(j + 1) * DIM]
                        rhs = x[:, c * BCHUNK:(c + 1) * BCHUNK] if j < 2 \
                            else x[:, BSL + c * BCHUNK:BSL + (c + 1) * BCHUNK]
                        nc.tensor.matmul(cat_ps[:, j, :], lhsT, rhs,
                                         start=True, stop=True)
                    cat_sb = apool.tile([DIM, 4, BCHUNK], F32, tag="cat_sb")
                    for j in range(4):
                        nc.vector.tensor_scalar_add(
                            cat_sb[:, j, :], cat_ps[:, j, :],
                            w[:, 2 * HID + 4 * DIM + j:
                              2 * HID + 4 * DIM + j + 1])
                    out_ps = pout.tile([DIM, BCHUNK], F32, tag="out_ps")
                    for j in range(4):
                        nc.tensor.matmul(
                            out_ps[:],
                            w[:, 2 * HID + j * DIM:2 * HID + (j + 1) * DIM],
                            cat_sb[:, j, :], start=(j == 0), stop=(j == 3))
                    nc.vector.tensor_scalar_add(out_sb[:, bs], out_ps[:],
                                                w[:, WCOLS - 1:WCOLS])
                nc.sync.dma_start(out=io[f"{s}_updT"][:], in_=out_sb[:])

            compute_side("src")
            for p in range(9, CP_NPAIRS):
                copy_pair("src", p)
                copy_pair("dst", p)
            compute_side("dst")
            copy_rem("src")
            copy_rem("dst")

    _split_multi_waits(nc)
    return nc


def _get_nc():
    if "nc" not in _CACHE:
        _CACHE["nc"] = _build_nc()
    return _CACHE["nc"]


def _f32(x):
    return np.ascontiguousarray(np.asarray(x), dtype=np.float32)


def kernel(**inputs):
    nc = _get_nc()

    prev = {s: _f32(inputs[f"{s}_previous_embedding"]) for s in SIDES}
    nig = {s: _f32(inputs[f"batch_{s}_neighbor_embedding"]) for s in SIDES}
    ids = {s: np.asarray(inputs[f"{s}_node_ids"]).astype(np.int64)
           for s in SIDES}
    wcat = {}
    for s in SIDES:
        b_res = _f32(inputs[f"b_{s}_resize"])
        b_nig = _f32(inputs[f"b_{s}_nig"])
        # wout [512,128] -> [k=128, 4*128]: col (c*128+d) = W[c*128+k, d]
        wout = _f32(inputs[f"W_{s}_out"]).reshape(4, DIM, DIM) \
            .transpose(1, 0, 2).reshape(DIM, 4 * DIM)
        bhid = np.stack([b_res[:DIM], b_res[DIM:],
                         b_nig[:DIM], b_nig[DIM:]], axis=1)
        wcat[s] = np.ascontiguousarray(np.concatenate(
            [_f32(inputs[f"W_{s}_resize"]), _f32(inputs[f"W_{s}_nig"]),
             wout, bhid, _f32(inputs[f"b_{s}_out"])[:, None]], axis=1))

    in_maps = []
    for i in range(N_CORES):
        m = {}
        bsl = slice(BSL * i, BSL * (i + 1))
        for s in SIDES:
            m[f"{s}_shard"] = prev[s][
                BATCH + ROWS * i:BATCH + ROWS * (i + 1)].reshape(-1)
            xT = np.concatenate([prev[s][ids[s][bsl]], nig[s][bsl]],
                                axis=0).T
            m[f"{s}_ins"] = np.ascontiguousarray(
                np.concatenate([wcat[s], xT], axis=1))
        in_maps.append(m)

    res = run_bass_kernel_spmd(nc, in_maps, list(range(N_CORES))).results

    outs = []
    for s in SIDES:
        out = np.empty((N_NODES, DIM), np.float32)
        out[:BATCH] = prev[s][:BATCH]
        for i in range(N_CORES):
            out[BATCH + ROWS * i:BATCH + ROWS * (i + 1)] = \
                res[i][f"{s}_out_shard"].reshape(ROWS, DIM)
        upd = np.concatenate(
            [res[i][f"{s}_updT"].T for i in range(N_CORES)], axis=0)
        out[ids[s]] = upd
        outs.append(out)
    return tuple(outs)


# revision 28
# speedup vs baseline: 1.3638x; 1.3638x over previous
"""Trainium2 Bass kernel for BatchEmbeddingUpdater (GNN message passing).

Contract: kernel(**inputs) takes the FULL inputs (as produced by the
reference setup_inputs()) and returns the FULL outputs
(updated_src_table, updated_dst_table), each [200000, 128] f32.

Sharding strategy (8 cores):
  - Both node-embedding tables are sharded row-block-wise over the
    non-updated region [BATCH, N_NODES); each core copies its shard
    input->output on device (HBM->HBM DMA) - the memory-bound bulk.
    The batch rows' old values reach the device as gather inputs and
    their new values come back as compute outputs, so copying them too
    would be redundant traffic.
  - The 8192-row batch is sharded by batch position: core i computes batch
    rows [1024*i, 1024*(i+1)) for BOTH sides. The host routes the gathered
    previous-embedding rows for those batch positions to core i (pre
    transposed to [128, 1024] so the device needs no transposes), the core
    runs the two-layer MLP, and returns the updated rows transposed
    [128, 1024]. The host scatters them into the assembled output.
  - The small linear weights are replicated to every core (packed into a
    single [128, 1029] tensor per side so one DMA loads them).

All DMA rides the sync (SP) HWDGE ring in FIFO order: the 4 input loads
first, then the shard-copy chunks with the updT stores interleaved
between them so the stores drain mid-stream instead of behind 24.5MB of
copy descriptors.
"""

import numpy as np

import concourse.bass as bass
import concourse.tile as tile
from concourse import mybir
from concourse.bass_utils import run_bass_kernel_spmd


def _split_multi_waits(nc, max_waits=1):
    """The walrus build in this image rejects multiple sem waits on one
    instruction ("Too many sync wait commands"). Move excess waits onto
    single-wait NOPs inserted just before the instruction on the same
    engine (per-engine program order makes this equivalent)."""
    ctr = 0
    for fn in nc.m.functions:
        for blk in fn.blocks:
            new_insts = []
            changed = False
            for ins in blk.instructions:
                si = ins.sync_info
                waits = list(si.on_wait) if si is not None else []
                if len(waits) > max_waits:
                    changed = True
                    for i in range(max_waits, len(waits), max_waits):
                        nop = mybir.InstNoOp(
                            name=f"I-waitsplit-{ctr}",
                            engine=ins.engine,
                            sync_info=mybir.SyncInfo(
                                on_wait=waits[i:i + max_waits], on_update=[]),
                        )
                        ctr += 1
                        new_insts.append(nop)
                    ins.sync_info = mybir.SyncInfo(
                        on_wait=waits[:max_waits],
                        on_update=list(si.on_update))
                new_insts.append(ins)
            if changed:
                blk.instructions = new_insts


N_CORES = 8
N_NODES = 200000
BATCH = 8192
ROWS = (N_NODES - BATCH) // N_CORES  # 23976 copied rows per core
DIM = 128                  # node/nig embedding dim
HID = 256                  # hidden dim
BSL = BATCH // N_CORES     # 1024 batch rows per core
BCHUNK = 512               # batch columns per matmul (one PSUM bank)
WCOLS = 2 * HID + 4 * DIM + 4 + 1  # packed weights: 1029 cols

# Shard-copy descriptor scheme. SDMA engine slot 15 of the HWDGE ring
# runs ~18% slower than the other 15 (queue bookkeeping rides its AXI
# path), and each DMA's descriptors are dealt to engine slots starting
# from slot 0, so slot 15 only sees descriptor 16 of a 16-desc DMA.
# A uniform byte split therefore leaves slot 15 straggling ~10us after
# everyone else. Mix: ~83% of copy bytes ride 16-desc DMAs (all engines)
# and ~17% ride 15-desc DMAs (slot 15 excluded; descriptor size 16001 is
# prime, which forces the splitter to exactly 15 descriptors), matching
# each engine's share to its capacity.
CP_A = 256000              # elems per 16-desc chunk (descs of 64000B)
CP_NA = 10                 # 16-desc chunks per table side
CP_B = 15 * 16001          # elems per 15-desc chunk (240015)
CP_NB = 2                  # 15-desc chunks per table side
CP_REM = ROWS * DIM - CP_NA * CP_A - CP_NB * CP_B  # 28898 elems (2 descs)

F32 = mybir.dt.float32
SIDES = ("src", "dst")

_CACHE: dict = {}


def _build_nc():
    nc = bass.Bass("TRN2", target_bir_lowering=False, debug=False,
                   num_devices=N_CORES)

    io = {}
    for s in SIDES:
        io[f"{s}_shard"] = nc.dram_tensor(
            f"{s}_shard", [ROWS * DIM], F32, kind="ExternalInput").ap()
        io[f"{s}_ins"] = nc.dram_tensor(
            f"{s}_ins", [DIM, WCOLS + 2 * BSL], F32,
            kind="ExternalInput").ap()
        io[f"{s}_out_shard"] = nc.dram_tensor(
            f"{s}_out_shard", [ROWS * DIM], F32, kind="ExternalOutput").ap()
        io[f"{s}_updT"] = nc.dram_tensor(
            f"{s}_updT", [DIM, BSL], F32, kind="ExternalOutput").ap()

    # chunk offsets per side: CP_NA A-chunks, then CP_NB B-chunks, then rem
    cp_slices = []
    o = 0
    for _ in range(CP_NA):
        cp_slices.append((o, o + CP_A))
        o += CP_A
    for _ in range(CP_NB):
        cp_slices.append((o, o + CP_B))
        o += CP_B
    cp_slices.append((o, o + CP_REM))

    def copy_chunk(s, idx):
        a, b = cp_slices[idx]
        nc.sync.dma_start(out=io[f"{s}_out_shard"][a:b],
                          in_=io[f"{s}_shard"][a:b])

    with tile.TileContext(nc) as tc:
        with (
            tc.tile_pool(name="const", bufs=1) as cpool,
            tc.tile_pool(name="acts", bufs=2) as apool,
            tc.tile_pool(name="outs", bufs=4) as opool,
            tc.tile_pool(name="psum_cat", bufs=1, space="PSUM") as pcat,
            tc.tile_pool(name="psum_out", bufs=2, space="PSUM") as pout,
        ):
            cons = {}
            for s in SIDES:
                t = cpool.tile([DIM, WCOLS + 2 * BSL], F32, tag=f"{s}_ins")
                nc.sync.dma_start(out=t[:], in_=io[f"{s}_ins"][:])
                cons[f"{s}_ins"] = t

            # feed the ring: most chunks up front, the rest after the
            # store stalls so the stream never runs dry. Interleave the
            # 15-desc B-chunks (indices CP_NA..) among the A-chunks so
            # slot-15 idle time is spread across the stream.
            order = [0, 1, 2, CP_NA, 3, 4, 5, CP_NA + 1, 6, 7, 8, 9]
            for idx in order[:10]:
                copy_chunk("src", idx)
                copy_chunk("dst", idx)

            def compute_side(s):
                w = cons[f"{s}_ins"][:, :WCOLS]
                x = cons[f"{s}_ins"][:, WCOLS:]
                out_sb = opool.tile([DIM, BSL], F32, tag="out_sb")
                for c in range(BSL // BCHUNK):
                    bs = bass.ts(c, BCHUNK)
                    # catT chunks: [sel0, sel1, shift0, shift1];
                    # chunk j covers hidden units [128j, 128(j+1))
                    cat_ps = pcat.tile([DIM, 4, BCHUNK], F32, tag="cat")
                    for j in range(4):
                        lhsT = w[:, j * DIM:(j + 1) * DIM]
                        rhs = x[:, c * BCHUNK:(c + 1) * BCHUNK] if j < 2 \
                            else x[:, BSL + c * BCHUNK:BSL + (c + 1) * BCHUNK]
                        nc.tensor.matmul(cat_ps[:, j, :], lhsT, rhs,
                                         start=True, stop=True)
                    cat_sb = apool.tile([DIM, 4, BCHUNK], F32, tag="cat_sb")
                    for j in range(4):
                        nc.vector.tensor_scalar_add(
                            cat_sb[:, j, :], cat_ps[:, j, :],
                            w[:, 2 * HID + 4 * DIM + j:
                              2 * HID + 4 * DIM + j + 1])
                    out_ps = pout.tile([DIM, BCHUNK], F32, tag="out_ps")
                    for j in range(4):
                        nc.tensor.matmul(
                            out_ps[:],
                            w[:, 2 * HID + j * DIM:2 * HID + (j + 1) * DIM],
                            cat_sb[:, j, :], start=(j == 0), stop=(j == 3))
                    nc.vector.tensor_scalar_add(out_sb[:, bs], out_ps[:],
                                                w[:, WCOLS - 1:WCOLS])
                nc.sync.dma_start(out=io[f"{s}_updT"][:], in_=out_sb[:])

            compute_side("src")
            for idx in order[10:]:
                copy_chunk("src", idx)
                copy_chunk("dst", idx)
            compute_side("dst")
            copy_chunk("src", CP_NA + CP_NB)
            copy_chunk("dst", CP_NA + CP_NB)

    _split_multi_waits(nc)
    return nc


def _get_nc():
    if "nc" not in _CACHE:
        _CACHE["nc"] = _build_nc()
    return _CACHE["nc"]


def _f32(x):
    return np.ascontiguousarray(np.asarray(x), dtype=np.float32)


def kernel(**inputs):
    nc = _get_nc()

    prev = {s: _f32(inputs[f"{s}_previous_embedding"]) for s in SIDES}
    nig = {s: _f32(inputs[f"batch_{s}_neighbor_embedding"]) for s in SIDES}
    ids = {s: np.asarray(inputs[f"{s}_node_ids"]).astype(np.int64)
           for s in SIDES}
    wcat = {}
    for s in SIDES:
        b_res = _f32(inputs[f"b_{s}_resize"])
        b_nig = _f32(inputs[f"b_{s}_nig"])
        # wout [512,128] -> [k=128, 4*128]: col (c*128+d) = W[c*128+k, d]
        wout = _f32(inputs[f"W_{s}_out"]).reshape(4, DIM, DIM) \
            .transpose(1, 0, 2).reshape(DIM, 4 * DIM)
        bhid = np.stack([b_res[:DIM], b_res[DIM:],
                         b_nig[:DIM], b_nig[DIM:]], axis=1)
        wcat[s] = np.ascontiguousarray(np.concatenate(
            [_f32(inputs[f"W_{s}_resize"]), _f32(inputs[f"W_{s}_nig"]),
             wout, bhid, _f32(inputs[f"b_{s}_out"])[:, None]], axis=1))

    in_maps = []
    for i in range(N_CORES):
        m = {}
        bsl = slice(BSL * i, BSL * (i + 1))
        for s in SIDES:
            m[f"{s}_shard"] = prev[s][
                BATCH + ROWS * i:BATCH + ROWS * (i + 1)].reshape(-1)
            xT = np.concatenate([prev[s][ids[s][bsl]], nig[s][bsl]],
                                axis=0).T
            m[f"{s}_ins"] = np.ascontiguousarray(
                np.concatenate([wcat[s], xT], axis=1))
        in_maps.append(m)

    res = run_bass_kernel_spmd(nc, in_maps, list(range(N_CORES))).results

    outs = []
    for s in SIDES:
        out = np.empty((N_NODES, DIM), np.float32)
        out[:BATCH] = prev[s][:BATCH]
        for i in range(N_CORES):
            out[BATCH + ROWS * i:BATCH + ROWS * (i + 1)] = \
                res[i][f"{s}_out_shard"].reshape(ROWS, DIM)
        upd = np.concatenate(
            [res[i][f"{s}_updT"].T for i in range(N_CORES)], axis=0)
        out[ids[s]] = upd
        outs.append(out)
    return tuple(outs)
